# revision 12
# baseline (speedup 1.0000x reference)
"""DeformConv3D on 8 TRN2 cores: H-sharded, dense 5-tap tent-weight gather.

Per core (h-band of 12 output rows + halos):
  P1: offset conv (27 taps, K=64 matmuls accumulated in PSUM) -> off scratch DRAM
  P2: trilinear gather as separable 5-tap tent-weighted sums on DVE
      (one (b,c) plane per partition; all shifts are AP offsets into a
       padded per-plane window; tent weights vanish outside the clamp range
       so padded reads are weight-zero)
  P3: main conv + bias -> output h-band
"""
import sys, os
import numpy as np
from contextlib import ExitStack

sys.path.insert(0, "/opt/trn_rl_repo")
from concourse import bass, bacc, tile, mybir
from concourse.bass_utils import run_bass_kernel_spmd

F32 = mybir.dt.float32
BF16 = mybir.dt.bfloat16
ALU = mybir.AluOpType
AF = mybir.ActivationFunctionType

B, C, L, H, W = 2, 64, 16, 96, 96
CO1, CO2 = 192, 64
NCORES = 8
HB = H // NCORES       # 12 output rows per core
HW_ROWS = 20           # x window rows per core: [12k-4, 12k+16)
HG = 14                # gather rows per core: [12k-1, 12k+13)
NPP = HG * W           # 1344 gather outputs per (plane, l)
ZPAD, XPAD = 20, 100   # gather window padded dims (taps +-2)
WIN = HW_ROWS * ZPAD * XPAD
CZP, CXP = 18, 98      # conv window padded dims (taps +-1)
TAPS = (-2, -1, 0, 1, 2)

_nc1_cache = None
_nc2_cache = None


def build_program1():
    nc = bacc.Bacc("TRN2", target_bir_lowering=False, debug=False, num_devices=NCORES)
    xwin = nc.dram_tensor("xwin", [B, C, L, HW_ROWS, W], F32, kind="ExternalInput").ap()
    w_off = nc.dram_tensor("w_off", [27, C, CO1], F32, kind="ExternalInput").ap()
    off_scr = nc.dram_tensor("off_band", [B, CO1, L, HG, W], F32, kind="ExternalOutput").ap()
    ctx = ExitStack()
    with tile.TileContext(nc) as tc:
        # ---------------- Phase 1: offset conv ----------------
        with tc.tile_pool(name="p1", bufs=1) as p1, \
             tc.tile_pool(name="p1ps", bufs=2, space="PSUM") as p1ps, \
             tc.tile_pool(name="p1o", bufs=3) as p1o:
            wofft = p1.tile([C, 27, CO1], F32)
            nc.sync.dma_start(wofft[:], w_off.rearrange("t c m -> c t m"))
            for b in range(B):
                xc = p1.tile([C, CZP, HW_ROWS, CXP], F32, tag="xc")
                nc.vector.memset(xc[:].rearrange("c z y x -> c (z y x)"), 0.0)
                for z in range(L):
                    nc.sync.dma_start(xc[:, 1 + z, :, 1:W + 1], xwin[b, :, z])
                for l in range(L):
                    for hc0, hcn in ((0, 5), (5, 5), (10, 4)):
                        nmm = hcn * W
                        for m0, mw in ((0, 128), (128, 64)):
                            ps = p1ps.tile([128, 480], F32, tag="ps1")
                            for t in range(27):
                                dz, rem = divmod(t, 9)
                                dy, dx = divmod(rem, 3)
                                rhs = xc[:, l + dz,
                                         2 + hc0 + dy:2 + hc0 + dy + hcn,
                                         dx:dx + W]
                                nc.tensor.matmul(
                                    ps[:mw, :nmm], wofft[:, t, m0:m0 + mw],
                                    rhs, start=(t == 0), stop=(t == 26))
                            ob = p1o.tile([128, 480], F32, tag="ob1")
                            nc.vector.tensor_copy(ob[:mw, :nmm], ps[:mw, :nmm])
                            nc.sync.dma_start(
                                off_scr[b, m0:m0 + mw, l, hc0:hc0 + hcn, :]
                                .rearrange("m h x -> m (h x)"),
                                ob[:mw, :nmm])

    nc.finalize()
    return nc


def build_program2():
    nc = bacc.Bacc("TRN2", target_bir_lowering=False, debug=False, num_devices=NCORES)
    xwin = nc.dram_tensor("xwin", [B, C, L, HW_ROWS, W], F32, kind="ExternalInput").ap()
    w_conv = nc.dram_tensor("w_conv", [27, C, CO2], F32, kind="ExternalInput").ap()
    b_conv = nc.dram_tensor("b_conv", [CO2, 1], F32, kind="ExternalInput").ap()
    offs = nc.dram_tensor("offs", [128, 3, L, NPP], F32, kind="ExternalInput").ap()
    grids = nc.dram_tensor("grids", [128, 3, NPP], F32, kind="ExternalInput").ap()
    params = nc.dram_tensor("params", [128, 2], F32, kind="ExternalInput").ap()
    out_ext = nc.dram_tensor("out", [B, CO2, L, HB, W], F32, kind="ExternalOutput").ap()
    def_scr = nc.dram_tensor("def_scr", [B, C, L, HG, W], F32).ap()
    ctx = ExitStack()
    with tile.TileContext(nc) as tc:
        # ---------------- Phase 2: tent gather ----------------
        with tc.tile_pool(name="p2w", bufs=1) as p2w, \
             tc.tile_pool(name="p2", bufs=1) as p2:
            win = p2w.tile([128, HW_ROWS, ZPAD, XPAD], BF16)
            nc.vector.memset(win[:].rearrange("p y z x -> p (y z x)"), 0.0)
            for b in range(B):
                for z in range(L):
                    nc.gpsimd.dma_start(
                        win[64 * b:64 * b + 64, :, 2 + z, 2:W + 2],
                        xwin[b, :, z])
            gr = p2w.tile([128, 3, NPP], F32)
            nc.sync.dma_start(gr[:], grids)
            par = p2w.tile([128, 2], F32)
            nc.sync.dma_start(par[:], params)

            for l in range(L):
                offc = p2.tile([128, 3, NPP], F32, tag="off")
                nc.sync.dma_start(offc[:], offs[:, :, l, :])
                # displacements a = clamp(off + g, lo, hi) - g   (f32)
                az = p2.tile([128, NPP], F32, tag="az")
                nc.vector.tensor_scalar(az[:], offc[:, 0], float(l), 0.0, ALU.add, ALU.max)
                nc.vector.tensor_scalar(az[:], az[:], 15.0, -float(l), ALU.min, ALU.add)
                ay = p2.tile([128, NPP], F32, tag="ay")
                nc.vector.tensor_tensor(ay[:], offc[:, 1], gr[:, 0], ALU.add)
                nc.vector.tensor_scalar(ay[:], ay[:], par[:, 0:1], par[:, 1:2], ALU.max, ALU.min)
                nc.vector.tensor_tensor(ay[:], ay[:], gr[:, 0], ALU.subtract)
                ax = p2.tile([128, NPP], F32, tag="ax")
                nc.vector.tensor_tensor(ax[:], offc[:, 2], gr[:, 1], ALU.add)
                nc.vector.tensor_scalar(ax[:], ax[:], 0.0, 95.0, ALU.max, ALU.min)
                nc.vector.tensor_tensor(ax[:], ax[:], gr[:, 1], ALU.subtract)

                # tent weights lam[dim][tap] = relu(1 - |a - t|)  (bf16)
                tneg = p2.tile([128, NPP], F32, tag="tneg")
                tpos = p2.tile([128, NPP], F32, tag="tpos")

                def tents(a, dst_tag, taps):
                    row = []
                    for t in taps:
                        nc.vector.tensor_scalar(tpos[:], a[:], -float(t), None, ALU.add)
                        nc.vector.tensor_scalar(tneg[:], a[:], -1.0, float(t), ALU.mult, ALU.add)
                        nc.vector.tensor_tensor(tpos[:], tpos[:], tneg[:], ALU.max)
                        lt = p2.tile([128, NPP], BF16, tag=f"{dst_tag}_{t}")
                        nc.vector.tensor_scalar(lt[:], tpos[:], -1.0, 1.0, ALU.mult, ALU.add)
                        nc.vector.tensor_scalar(lt[:], lt[:], 0.0, None, ALU.max)
                        row.append(lt)
                    return row

                lamx = tents(ax, "lamx", TAPS)
                lamy = tents(ay, "lamy", TAPS)

                acc = p2.tile([128, NPP], F32, tag="acc")
                tmpi = p2.tile([128, NPP], F32, tag="tmpi")
                tmpb = p2.tile([128, NPP], F32, tag="tmpb")
                prod = p2.tile([128, NPP], BF16, tag="prod")
                lam = [None, lamy, lamx]
                for iz, sz in enumerate(TAPS):
                    lamz = tents(az, "lamz", (sz,))[0]
                    for iy, sy in enumerate(TAPS):
                        for ix, sx in enumerate(TAPS):
                            v = win[:, 3 + sy:3 + sy + HG,
                                    l + 2 + sz,
                                    2 + sx:2 + sx + W]
                            nc.vector.tensor_tensor(prod[:], lam[2][ix][:], v, ALU.mult)
                            if ix == 0:
                                nc.vector.tensor_copy(tmpi[:], prod[:])
                            else:
                                nc.vector.tensor_tensor(tmpi[:], tmpi[:], prod[:], ALU.add)
                        if iy == 0:
                            nc.vector.tensor_tensor(tmpb[:], lam[1][0][:], tmpi[:], ALU.mult)
                        else:
                            nc.vector.tensor_tensor(tmpi[:], lam[1][iy][:], tmpi[:], ALU.mult)
                            nc.vector.tensor_tensor(tmpb[:], tmpb[:], tmpi[:], ALU.add)
                    if iz == 0:
                        nc.vector.tensor_tensor(acc[:], lamz[:], tmpb[:], ALU.mult)
                    else:
                        nc.vector.tensor_tensor(tmpb[:], lamz[:], tmpb[:], ALU.mult)
                        nc.vector.tensor_tensor(acc[:], acc[:], tmpb[:], ALU.add)
                # zero rows whose global h is outside [0, 96)
                nc.vector.tensor_tensor(acc[:], acc[:], gr[:, 2], ALU.mult)
                for b in range(B):
                    nc.sync.dma_start(
                        def_scr[b, :, l].rearrange("c h x -> c (h x)"),
                        acc[64 * b:64 * b + 64, :])

        # ---------------- Phase 3: main conv ----------------
        with tc.tile_pool(name="p3", bufs=1) as p3, \
             tc.tile_pool(name="p3ps", bufs=2, space="PSUM") as p3ps, \
             tc.tile_pool(name="p3o", bufs=3) as p3o:
            wct = p3.tile([C, 27, CO2], F32)
            nc.sync.dma_start(wct[:], w_conv.rearrange("t c m -> c t m"))
            bct = p3.tile([CO2, 1], F32)
            nc.sync.dma_start(bct[:], b_conv)
            for b in range(B):
                dc = p3.tile([C, CZP, HG + 2, CXP], F32, tag="dc")
                nc.vector.memset(dc[:].rearrange("c z y x -> c (z y x)"), 0.0)
                for z in range(L):
                    nc.sync.dma_start(dc[:, 1 + z, 1:HG + 1, 1:W + 1], def_scr[b, :, z])
                for l in range(L):
                    for hc0, hcn in ((0, 5), (5, 5), (10, 2)):
                        nmm = hcn * W
                        ps = p3ps.tile([CO2, 480], F32, tag="ps3")
                        for t in range(27):
                            dz, rem = divmod(t, 9)
                            dy, dx = divmod(rem, 3)
                            # out row r=4+hc0+j -> dc y index r+dy-3
                            rhs = dc[:, l + dz,
                                     1 + hc0 + dy:1 + hc0 + dy + hcn,
                                     dx:dx + W]
                            nc.tensor.matmul(
                                ps[:, :nmm], wct[:, t, :],
                                rhs, start=(t == 0), stop=(t == 26))
                        ob = p3o.tile([CO2, 480], F32, tag="ob3")
                        nc.vector.tensor_scalar(ob[:, :nmm], ps[:, :nmm], bct[:], None, ALU.add)
                        nc.sync.dma_start(
                            out_ext[b, :, l, hc0:hc0 + hcn, :]
                            .rearrange("m h x -> m (h x)"),
                            ob[:, :nmm])
    nc.finalize()
    return nc


def kernel(x, w_off, w_conv, b_conv):
    global _nc1_cache, _nc2_cache
    x = np.asarray(x, dtype=np.float32)
    w_off = np.asarray(w_off, dtype=np.float32)
    w_conv = np.asarray(w_conv, dtype=np.float32)
    b_conv = np.asarray(b_conv, dtype=np.float32)

    if _nc1_cache is None:
        _nc1_cache = build_program1()
        _nc2_cache = build_program2()

    xp = np.zeros((B, C, L, H + 8, W), np.float32)
    xp[:, :, :, 4:4 + H, :] = x
    wofft = np.ascontiguousarray(
        w_off.reshape(CO1, C, 27).transpose(2, 1, 0))        # [27, C, CO1]
    wct = np.ascontiguousarray(
        w_conv.reshape(CO2, C, 27).transpose(2, 1, 0))       # [27, C, CO2]
    bc = np.ascontiguousarray(b_conv.reshape(CO2, 1))

    xwins = [np.ascontiguousarray(xp[:, :, :, 12 * k:12 * k + HW_ROWS, :])
             for k in range(NCORES)]
    in1 = [{"xwin": xwins[k], "w_off": wofft} for k in range(NCORES)]
    res1 = run_bass_kernel_spmd(_nc1_cache, in1, list(range(NCORES)))

    # reassemble full off field from per-core bands (band rows = 12k-1..12k+13)
    off_full = np.empty((B, CO1, L, H, W), np.float32)
    for k in range(NCORES):
        band = res1.results[k]["off_band"]
        off_full[:, :, :, 12 * k:12 * k + HB, :] = band[:, :, :, 1:1 + HB, :]
    # the contiguous-view scramble, done on host: plane (b,c) offsets at
    # spatial p, comp k = flat element 3p+k of its 3-channel block
    triple = off_full.reshape(B, C, 3 * L * H * W)
    r = np.arange(HG, dtype=np.int64)                        # local gather rows
    in2 = []
    gy = np.repeat(np.arange(HG, dtype=np.float32) + 3.0, W)
    gx = np.tile(np.arange(W, dtype=np.float32), HG)
    for k in range(NCORES):
        hband = np.clip(np.arange(HG) + (12 * k - 1), 0, H - 1)   # clamp edge rows
        pflat = (np.arange(L)[:, None, None] * (H * W)
                 + hband[None, :, None] * W
                 + np.arange(W)[None, None, :]).reshape(L, NPP)
        idx3 = (3 * pflat[None, :, :]
                + np.arange(3, dtype=np.int64)[:, None, None])    # [3, L, NPP]
        offs = triple[:, :, idx3].reshape(128, 3, L, NPP)
        hglob = np.repeat(np.arange(HG) + (12 * k - 1), W)
        ymask = ((hglob >= 0) & (hglob < H)).astype(np.float32)
        grids = np.broadcast_to(
            np.stack([gy, gx, ymask])[None], (128, 3, NPP)).copy()
        hbase = 12 * k - 4
        par = np.zeros((128, 2), np.float32)
        par[:, 0] = float(-hbase)
        par[:, 1] = float(95 - hbase)
        in2.append({
            "xwin": xwins[k], "w_conv": wct, "b_conv": bc,
            "offs": np.ascontiguousarray(offs),
            "grids": grids, "params": par,
        })
    res2 = run_bass_kernel_spmd(_nc2_cache, in2, list(range(NCORES)))
    out = np.empty((B, CO2, L, H, W), np.float32)
    for k in range(NCORES):
        out[:, :, :, 12 * k:12 * k + HB, :] = res2.results[k]["out"]
    return out
